# revision 26
# baseline (speedup 1.0000x reference)
"""GQA attention block (B=2, S=2048, DIM=4096, 32 Q heads / 8 KV heads, HD=128,
RoPE + causal softmax + output projection) on 8 trn2 NeuronCores.

Sharding: 8 cores = 2 batches x 4 head-groups. Core c handles batch c%2 and
head-group c//2 (8 Q heads, 2 KV heads). Each core computes a full-size
[S, DIM] partial of the output projection (its heads' contribution); the host
sums the 4 group-partials per batch.

v4: v3 (hybrid fp32r/bf16, software-pipelined attention) + sync-engine relief.
The sync sequencer pays ~0.6us per dma_start; v3 lost ~38us at the A->Q seam
to a burst of single-chunk issues. v4:
  - batches every weight/x stream into multi-chunk dma_starts,
  - replaces the RoPE half-swap SBUF-SBUF DMA pair with one
    vector.stream_shuffle,
  - prefetches the first wq piece during phase A and the first wo slabs
    during phase S,
  - fuses the causal mask into one precomputed multiplicative bf16 tile per
    diagonal offset (single DVE op per diagonal key-tile).
"""

import math
import os
import sys
from contextlib import ExitStack
from dataclasses import dataclass

import numpy as np

sys.path.insert(0, "/opt/trn_rl_repo")

import concourse.bass as bass  # noqa: E402
import concourse.mybir as mybir  # noqa: E402
import concourse.tile as tile  # noqa: E402
from concourse import bacc  # noqa: E402

F32 = mybir.dt.float32
F32R = mybir.dt.float32r
BF16 = mybir.dt.bfloat16
P = 128

SWAP_MASK = list(range(16, 32)) + list(range(16))  # half-swap in 4-row groups


@dataclass(frozen=True)
class Cfg:
    S: int = 2048      # sequence length
    DIM: int = 4096    # model dim (contraction for projections)
    NH_L: int = 8      # q heads per core
    NKV_L: int = 2     # kv heads per core
    HD: int = 128      # head dim (must be P)
    TQ: int = 512      # token/query chunk (PSUM free dim)

    @property
    def CCH(self):  # contraction chunks
        return self.DIM // P

    @property
    def NT(self):  # token chunks
        return self.S // self.TQ

    @property
    def NKT(self):  # key tiles
        return self.S // P

    @property
    def RT(self):  # key tiles per token chunk
        return self.TQ // P

    @property
    def NREP(self):
        return self.NH_L // self.NKV_L


def build_program(cfg: Cfg, debug: bool = False) -> bass.Bass:
    nc = bacc.Bacc("TRN2", target_bir_lowering=False)
    S, DIM, NH_L, NKV_L, HD, TQ = cfg.S, cfg.DIM, cfg.NH_L, cfg.NKV_L, cfg.HD, cfg.TQ
    CCH, NT, RT = cfg.CCH, cfg.NT, cfg.RT
    MULT = mybir.AluOpType.mult

    xT_d = nc.dram_tensor("xT", [DIM, S], F32R, kind="ExternalInput")
    wq_d = nc.dram_tensor("wq", [DIM, NH_L * HD], F32R, kind="ExternalInput")
    wk_d = nc.dram_tensor("wk", [DIM, NKV_L * HD], F32R, kind="ExternalInput")
    wv_d = nc.dram_tensor("wv", [DIM, NKV_L * HD], F32R, kind="ExternalInput")
    wo_d = nc.dram_tensor("wo", [NH_L * HD, DIM], F32R, kind="ExternalInput")
    cosq_d = nc.dram_tensor("cosq", [P, S], F32, kind="ExternalInput")
    sinq_d = nc.dram_tensor("sinq", [P, S], F32, kind="ExternalInput")
    cosk_d = nc.dram_tensor("cosk", [P, S], F32, kind="ExternalInput")
    sink_d = nc.dram_tensor("sink", [P, S], F32, kind="ExternalInput")
    pmask_d = nc.dram_tensor("pmask", [P, RT, TQ], BF16, kind="ExternalInput")
    out_d = nc.dram_tensor("out", [S, DIM], F32, kind="ExternalOutput")

    if debug:
        dbg_kt = nc.dram_tensor("dbg_kt", [P, NKV_L, S], F32, kind="ExternalOutput")
        dbg_v = nc.dram_tensor("dbg_v", [P, cfg.NKT, NKV_L * HD], BF16,
                               kind="ExternalOutput")
        dbg_qt = nc.dram_tensor("dbg_qt", [P, NH_L, S], F32, kind="ExternalOutput")
        dbg_at = nc.dram_tensor("dbg_at", [P, NH_L, S], F32, kind="ExternalOutput")

    xT_r = xT_d.ap().rearrange("(co ci) t -> ci co t", ci=P)
    wq_r = wq_d.ap().rearrange("(co ci) d -> ci co d", ci=P)
    wk_r = wk_d.ap().rearrange("(co ci) d -> ci co d", ci=P)
    wv_r = wv_d.ap().rearrange("(co ci) d -> ci co d", ci=P)
    wo_r = wo_d.ap().rearrange("(dc p) m -> p dc m", p=P)

    def r(ap):
        return ap if ap.dtype == F32R else ap.bitcast(F32R)

    def mm(out, lhsT, rhs, start, stop):
        nc.tensor.matmul(out, r(lhsT), r(rhs), start=start, stop=stop)

    def mmb(out, lhsT, rhs, start, stop):
        nc.tensor.matmul(out, lhsT, rhs, start=start, stop=stop)

    with tile.TileContext(nc) as tc, ExitStack() as top:
        const = top.enter_context(tc.tile_pool(name="const", bufs=1))
        pmask_sb = const.tile([P, RT, TQ], BF16)
        ones_row = const.tile([P, P], BF16)

        kvp = top.enter_context(tc.tile_pool(name="kvp", bufs=1))
        KT_sb = kvp.tile([P, NKV_L, S], F32)
        V_sb = kvp.tile([P, cfg.NKT, NKV_L * HD], BF16)
        qtp = tc.alloc_tile_pool(name="qtp", bufs=1)
        qt_sb = qtp.tile([P, NH_L, S], F32)
        # first wq piece, prefetched during phase A's tail
        wq0p = tc.alloc_tile_pool(name="wq0p", bufs=1)
        PCH = 4  # c-chunks per wq piece
        wq_piece0 = wq0p.tile([P, PCH, NH_L * HD], F32R)

        def rope_inplace(dst, cos_sl, sin_sl, tmp_pool):
            # dst [P, n] f32 in SBUF: dst = dst*cos + swap_halves(dst)*sin
            n = dst.shape[-1]
            tmp = tmp_pool.tile([P, TQ], F32, tag="ropetmp", name="ropetmp")
            t = tmp[:, :n]
            nc.vector.stream_shuffle(t, dst, SWAP_MASK)
            nc.vector.tensor_tensor(t.bitcast(F32R), t, sin_sl, MULT)
            nc.vector.tensor_tensor(dst.bitcast(F32R), dst, cos_sl, MULT)
            nc.vector.tensor_add(dst.bitcast(F32R), dst, t)

        # ---------------- Phase A: K^T and V projections (+ RoPE on K) -----
        with ExitStack() as ctx:
            wkvp = ctx.enter_context(tc.tile_pool(name="wkvp", bufs=1))
            ktab = ctx.enter_context(tc.tile_pool(name="ktab", bufs=2))
            xap = ctx.enter_context(tc.tile_pool(name="xap", bufs=2))
            rtp = ctx.enter_context(tc.tile_pool(name="rtp", bufs=2))
            pka = ctx.enter_context(tc.tile_pool(name="pka", bufs=2, space="PSUM"))
            pva = ctx.enter_context(tc.tile_pool(name="pva", bufs=1, space="PSUM"))

            wk_sb = wkvp.tile([P, CCH, NKV_L * HD], F32R)
            wv_sb = wkvp.tile([P, CCH, NKV_L * HD], F32R)

            XB = 4  # x chunks per batched tile/dma
            for tn in range(NT):
                tsl = slice(tn * TQ, (tn + 1) * TQ)
                psk = [pka.tile([P, TQ], F32, tag=f"psk{d}", name=f"psk{d}")
                       for d in range(NKV_L)]
                psv = [pva.tile([P, NKV_L * HD], F32, tag=f"psv{j}", name=f"psv{j}")
                       for j in range(RT)]
                xt4 = None
                for c in range(CCH):
                    if c % XB == 0:
                        xt4 = xap.tile([P, XB, TQ], F32R, tag="xa", name="xa")
                        nc.sync.dma_start(xt4[:], xT_r[:, c:c + XB, tsl])
                        if tn == 0:
                            # JIT weights one batch ahead; tables after batch 0
                            cc = c + XB
                            if c == 0:
                                nc.sync.dma_start(wk_sb[:, 0:XB, :],
                                                  wk_r[:, 0:XB, :])
                                nc.sync.dma_start(wv_sb[:, 0:XB, :],
                                                  wv_r[:, 0:XB, :])
                                nc.sync.dma_start(pmask_sb[:], pmask_d.ap())
                                nc.vector.memset(ones_row[:], 1.0)
                            if cc < CCH:
                                nc.sync.dma_start(wk_sb[:, cc:cc + XB, :],
                                                  wk_r[:, cc:cc + XB, :])
                                nc.sync.dma_start(wv_sb[:, cc:cc + XB, :],
                                                  wv_r[:, cc:cc + XB, :])
                        if tn == NT - 1 and c == 0:
                            # prefetch the first wq piece for phase Q
                            nc.sync.dma_start(wq_piece0[:], wq_r[:, 0:PCH, :])
                    if c % XB == 1 and c // XB == 0:
                        # per-tn K rope tables (small, after the gating loads)
                        cosk_t = ktab.tile([P, TQ], F32, tag="ckt", name="ckt")
                        sink_t = ktab.tile([P, TQ], F32, tag="skt", name="skt")
                        nc.sync.dma_start(cosk_t[:], cosk_d.ap()[:, tsl])
                        nc.sync.dma_start(sink_t[:], sink_d.ap()[:, tsl])
                    xt = xt4[:, c % XB, :]
                    st, sp = c == 0, c == CCH - 1
                    for d in range(NKV_L):
                        mm(psk[d][:], wk_sb[:, c, d * HD:(d + 1) * HD], xt, st, sp)
                    for j in range(RT):
                        mm(psv[j][:], xt[:, j * P:(j + 1) * P], wv_sb[:, c, :], st, sp)
                for j in range(RT):
                    nc.scalar.copy(V_sb[:, tn * RT + j, :], psv[j][:])
                for d in range(NKV_L):
                    nc.scalar.copy(KT_sb[:, d, tsl].bitcast(F32R), psk[d][:])
                    rope_inplace(KT_sb[:, d, tsl], cosk_t[:], sink_t[:], rtp)

        # ---------------- Phase Q: Q^T projection (+ RoPE on Q) ------------
        # 2-level accumulation, NACC=2 groups of 16 c-chunks; wq pieces of 4
        # chunks stream JIT (one batched dma each), piece 0 already resident.
        NACC = 2
        GC = CCH // NACC          # c-chunks per accumulation group (16)
        NPC = GC // PCH           # pieces per group (4)
        with ExitStack() as ctx:
            wqp = ctx.enter_context(tc.tile_pool(name="wqp", bufs=4))
            qtab = ctx.enter_context(tc.tile_pool(name="qtab", bufs=2))
            xqp = ctx.enter_context(tc.tile_pool(name="xqp", bufs=2))
            rtq = ctx.enter_context(tc.tile_pool(name="rtq", bufs=2))
            pqa = ctx.enter_context(tc.tile_pool(name="pqa", bufs=1, space="PSUM"))

            XB = 4

            def fetch_piece(g, p):
                if g == 0 and p == 0:
                    return wq_piece0
                piece = wqp.tile([P, PCH, NH_L * HD], F32R, tag="wqs", name="wqs")
                c0 = g * GC + p * PCH
                nc.sync.dma_start(piece[:], wq_r[:, c0:c0 + PCH, :])
                return piece

            for g in range(NACC):
                pieces = [fetch_piece(g, p) for p in range(NPC)]
                for tn in range(NT):
                    tsl = slice(tn * TQ, (tn + 1) * TQ)
                    if g == NACC - 1:
                        cq = qtab.tile([P, TQ], F32, tag="cqt", name="cqt")
                        sq = qtab.tile([P, TQ], F32, tag="sqt", name="sqt")
                        nc.sync.dma_start(cq[:], cosq_d.ap()[:, tsl])
                        nc.sync.dma_start(sq[:], sinq_d.ap()[:, tsl])
                    psq = [pqa.tile([P, TQ], F32, tag=f"psq{h}", name=f"psq{h}")
                           for h in range(NH_L)]
                    xt4 = None
                    for ci in range(GC):
                        if ci % XB == 0:
                            xt4 = xqp.tile([P, XB, TQ], F32R, tag="xq", name="xq")
                            c0 = g * GC + ci
                            nc.sync.dma_start(xt4[:], xT_r[:, c0:c0 + XB, tsl])
                        piece = pieces[ci // PCH]
                        col = ci % PCH
                        xt = xt4[:, ci % XB, :]
                        st, sp = ci == 0, ci == GC - 1
                        for h in range(NH_L):
                            mm(psq[h][:], piece[:, col, h * HD:(h + 1) * HD],
                               xt, st, sp)
                    for h in range(NH_L):
                        if g == 0:
                            nc.scalar.copy(qt_sb[:, h, tsl].bitcast(F32R),
                                           psq[h][:])
                        else:
                            nc.vector.tensor_add(qt_sb[:, h, tsl].bitcast(F32R),
                                                 qt_sb[:, h, tsl], psq[h][:])
                        if g == NACC - 1:
                            rope_inplace(qt_sb[:, h, tsl], cq[:], sq[:], rtq)

        wq0p.release()

        if debug:
            nc.sync.dma_start(dbg_kt.ap(), KT_sb[:])
            nc.sync.dma_start(dbg_v.ap(), V_sb[:])
            nc.sync.dma_start(dbg_qt.ap(), qt_sb[:])

        # ---------------- Phase S: attention per head ----------------------
        # Software-pipelined: PV(kt) emitted after scores(kt+LP); single fused
        # multiplicative bf16 mask per diagonal key-tile; denominator epilogue
        # (ones_row matmul -> broadcast sums in PSUM -> reciprocal ->
        # normalize) deferred one block. wo slabs for phase W prefetch here.
        atp = tc.alloc_tile_pool(name="atp", bufs=1, side="right")
        attnT_sb = atp.tile([P, NH_L, S], F32)
        wop = tc.alloc_tile_pool(name="wop", bufs=4, side="right")

        def fetch_wo(mc, dh):
            slab = wop.tile([P, 4, TQ], F32R, tag="wos", name="wos")
            msl = slice(mc * TQ, (mc + 1) * TQ)
            nc.sync.dma_start(slab[:], wo_r[:, dh * 4:dh * 4 + 4, msl])
            return slab

        with ExitStack() as ctx:
            ptp = ctx.enter_context(tc.tile_pool(name="ptp", bufs=6))
            acp = ctx.enter_context(tc.tile_pool(name="acp", bufs=3))
            bcp = ctx.enter_context(tc.tile_pool(name="bcp", bufs=2))
            psc = ctx.enter_context(tc.tile_pool(name="psc", bufs=4, space="PSUM"))
            pso = ctx.enter_context(tc.tile_pool(name="pso", bufs=3, space="PSUM"))
            pss = ctx.enter_context(tc.tile_pool(name="pss", bufs=1, space="PSUM"))

            LP = 3  # PV lookahead (score-matmuls emitted ahead of each PV)

            def epilogue(pend):
                h, qsl, ps_out, acc = pend
                ps_sum = pss.tile([P, TQ], F32, tag="pssum", name="pssum")
                mmb(ps_sum[:], ones_row[:], acc[:], True, True)
                bc_sb = bcp.tile([P, TQ], F32, tag="bcsb", name="bcsb")
                nc.vector.reciprocal_approx_fast(out=bc_sb[:], in_=ps_sum[:])
                nc.vector.tensor_tensor(attnT_sb[:, h, qsl].bitcast(F32R),
                                        ps_out[:], bc_sb[:], MULT)

            wo_fetched = []
            pending = None
            nblk = 0
            for qc in range(NT):
                for h in range(NH_L):
                    nblk += 1
                    if nblk >= NT * NH_L - 1 and len(wo_fetched) < 2:
                        wo_fetched.append(fetch_wo(0, len(wo_fetched)))
                    g = h // cfg.NREP
                    qsl = slice(qc * TQ, (qc + 1) * TQ)
                    ps_out = pso.tile([P, TQ], F32, tag="psout", name="psout")
                    acc = acp.tile([P, TQ], BF16, tag="acc", name="acc")
                    nkt = (qc + 1) * RT
                    pts = {}
                    for step in range(nkt + LP):
                        if step < nkt:
                            kt = step
                            ps_sc = psc.tile([P, TQ], F32, tag="pssc", name="pssc")
                            mm(ps_sc[:], KT_sb[:, g, kt * P:(kt + 1) * P],
                               qt_sb[:, h, qsl], True, True)
                            pt = ptp.tile([P, TQ], BF16, tag="pt", name="pt")
                            nc.scalar.activation(pt[:], ps_sc[:],
                                                 mybir.ActivationFunctionType.Exp)
                            if kt >= qc * RT:
                                j = kt - qc * RT
                                nc.vector.tensor_tensor(pt[:], pt[:],
                                                        pmask_sb[:, j, :], MULT)
                            if kt == 0:
                                nc.vector.tensor_copy(acc[:], pt[:])
                            else:
                                nc.vector.tensor_add(acc[:], acc[:], pt[:])
                            pts[kt] = pt
                        if step == min(2, nkt - 1) and pending is not None:
                            epilogue(pending)
                            pending = None
                        j = step - LP
                        if 0 <= j < nkt:
                            mmb(ps_out[:], V_sb[:, j, g * HD:(g + 1) * HD],
                                pts.pop(j)[:], j == 0, j == nkt - 1)
                    pending = (h, qsl, ps_out, acc)
            epilogue(pending)

        if debug:
            nc.sync.dma_start(dbg_at.ap(), attnT_sb[:])

        qtp.release()

        # ---------------- Phase W: output projection -----------------------
        with ExitStack() as ctx:
            owp = ctx.enter_context(tc.tile_pool(name="owp", bufs=3, side="right"))
            psw = ctx.enter_context(tc.tile_pool(name="psw", bufs=4, space="PSUM"))

            cur = wo_fetched
            for mc in range(DIM // TQ):
                msl = slice(mc * TQ, (mc + 1) * TQ)
                nxt = []
                for tb in range(S // P):
                    if mc + 1 < DIM // TQ and tb in (1, 3):
                        nxt.append(fetch_wo(mc + 1, len(nxt)))
                    ps_w = psw.tile([P, TQ], F32, tag="psw", name="psw")
                    for dc in range(NH_L):
                        mm(ps_w[:], attnT_sb[:, dc, tb * P:(tb + 1) * P],
                           cur[dc // 4][:, dc % 4, :], dc == 0, dc == NH_L - 1)
                    ot = owp.tile([P, TQ], F32, tag="ot", name="ot")
                    nc.scalar.copy(ot[:], ps_w[:])
                    nc.sync.dma_start(out_d.ap()[tb * P:(tb + 1) * P, msl], ot[:])
                if nxt:
                    cur = nxt

        wop.release()
        atp.release()

    nc.compile()
    return nc


# ---------------------------------------------------------------------------
# Host side
# ---------------------------------------------------------------------------

# Rope-pair layout: within each 32-partition quadrant q, rows q*32+[0:16) hold
# the even pair elements for frequency indices 16q+j and rows q*32+[16:32)
# the odd ones, so the rope partner swap is quadrant-local (stream_shuffle
# can only permute within 32-partition quadrants).
_QUAD_PERM = np.concatenate([
    np.concatenate([np.arange(0, P, 2)[q * 16:(q + 1) * 16],
                    np.arange(1, P, 2)[q * 16:(q + 1) * 16]])
    for q in range(4)
])
# row -> rope frequency index, and the sin sign per row
_FREQ_IDX = np.concatenate([np.tile(np.arange(16) + 16 * q, 2) for q in range(4)])
_SIN_SGN = np.tile(np.concatenate([-np.ones(16), np.ones(16)]), 4).astype(np.float32)

LAST_EXEC_NS = None
LAST_RESULTS = None


def _host_prep(cfg: Cfg, x, wq, wk, wv, wo, freqs_cos, freqs_sin):
    """Build the 8 per-core input maps. Core c: batch c % 2, group c // 2."""
    import ml_dtypes
    BF = ml_dtypes.bfloat16

    B = x.shape[0]
    n_groups = wq.shape[1] // (cfg.NH_L * cfg.HD)
    hd = cfg.HD

    cosT = np.ascontiguousarray(freqs_cos.T.astype(np.float32))  # [HD/2, S]
    sinT = np.ascontiguousarray(freqs_sin.T.astype(np.float32))
    sc = np.float32(1.0 / math.sqrt(hd))
    cosq = np.ascontiguousarray(cosT[_FREQ_IDX] * sc)
    sinq = np.ascontiguousarray(sinT[_FREQ_IDX] * _SIN_SGN[:, None] * sc)
    cosk = np.ascontiguousarray(cosT[_FREQ_IDX])
    sink = np.ascontiguousarray(sinT[_FREQ_IDX] * _SIN_SGN[:, None])
    # fused multiplicative causal masks for the transposed diagonal tiles:
    # variant j ([P, TQ]): cols < j*P -> 0, cols in [j*P,(j+1)*P) -> triu,
    # cols >= (j+1)*P -> 1   (allowed iff key k <= query q)
    pmask = np.ones((cfg.RT, P, cfg.TQ), np.float32)
    tri = np.triu(np.ones((P, P), np.float32), 0)
    for j in range(cfg.RT):
        pmask[j, :, : j * P] = 0.0
        pmask[j, :, j * P:(j + 1) * P] = tri
    pmask = np.ascontiguousarray(pmask.transpose(1, 0, 2)).astype(BF)  # [P,RT,TQ]

    xT = [np.ascontiguousarray(x[b].T).astype(np.float32) for b in range(B)]

    def permute_cols(w, nheads):
        w = w.reshape(cfg.DIM, nheads, hd)[:, :, _QUAD_PERM]
        return np.ascontiguousarray(w.reshape(cfg.DIM, nheads * hd), dtype=np.float32)

    in_maps = []
    qcols = cfg.NH_L * hd
    kcols = cfg.NKV_L * hd
    for c in range(B * n_groups):
        b, g = c % B, c // B
        in_maps.append(dict(
            xT=xT[b],
            wq=permute_cols(wq[:, g * qcols:(g + 1) * qcols], cfg.NH_L),
            wk=permute_cols(wk[:, g * kcols:(g + 1) * kcols], cfg.NKV_L),
            wv=np.ascontiguousarray(wv[:, g * kcols:(g + 1) * kcols], dtype=np.float32),
            wo=np.ascontiguousarray(wo[g * qcols:(g + 1) * qcols, :], dtype=np.float32),
            cosq=cosq, sinq=sinq, cosk=cosk, sink=sink, pmask=pmask,
        ))
    return in_maps


def kernel(x, wq, wk, wv, wo, freqs_cos, freqs_sin, mask, start_pos=0):
    global LAST_EXEC_NS, LAST_RESULTS
    x = np.asarray(x, np.float32)
    wq = np.asarray(wq, np.float32)
    wk = np.asarray(wk, np.float32)
    wv = np.asarray(wv, np.float32)
    wo = np.asarray(wo, np.float32)
    freqs_cos = np.asarray(freqs_cos, np.float32)
    freqs_sin = np.asarray(freqs_sin, np.float32)

    cfg = Cfg()
    B = x.shape[0]
    n_groups = 4
    in_maps = _host_prep(cfg, x, wq, wk, wv, wo, freqs_cos, freqs_sin)

    from concourse.bass_utils import run_bass_kernel_spmd

    debug = bool(int(os.environ.get("KERNEL_DEBUG", "0")))
    nc = build_program(cfg, debug=debug)
    trace = bool(int(os.environ.get("KERNEL_TRACE", "0")))
    res = run_bass_kernel_spmd(nc, in_maps, core_ids=list(range(len(in_maps))),
                               trace=trace)
    LAST_EXEC_NS = res.exec_time_ns
    LAST_RESULTS = res

    out = np.zeros((B, cfg.S, cfg.DIM), np.float32)
    for c in range(B * n_groups):
        b = c % B
        out[b] += res.results[c]["out"]
    return out


# revision 31
# speedup vs baseline: 1.0684x; 1.0684x over previous
"""GQA attention block (B=2, S=2048, DIM=4096, 32 Q heads / 8 KV heads, HD=128,
RoPE + causal softmax + output projection) on 8 trn2 NeuronCores.

Sharding: 8 cores = 2 batches x 4 head-groups. Core c handles batch c%2 and
head-group c//2 (8 Q heads, 2 KV heads). Each core computes a full-size
[S, DIM] partial of the output projection (its heads' contribution); the host
sums the 4 group-partials per batch.

v4: v3 (hybrid fp32r/bf16, software-pipelined attention) + sync-engine relief.
The sync sequencer pays ~0.6us per dma_start; v3 lost ~38us at the A->Q seam
to a burst of single-chunk issues. v4:
  - batches every weight/x stream into multi-chunk dma_starts,
  - replaces the RoPE half-swap SBUF-SBUF DMA pair with one
    vector.stream_shuffle,
  - prefetches the first wq piece during phase A and the first wo slabs
    during phase S,
  - fuses the causal mask into one precomputed multiplicative bf16 tile per
    diagonal offset (single DVE op per diagonal key-tile).
"""

import math
import os
import sys
from contextlib import ExitStack
from dataclasses import dataclass

import numpy as np

sys.path.insert(0, "/opt/trn_rl_repo")

import concourse.bass as bass  # noqa: E402
import concourse.mybir as mybir  # noqa: E402
import concourse.tile as tile  # noqa: E402
from concourse import bacc  # noqa: E402

F32 = mybir.dt.float32
F32R = mybir.dt.float32r
BF16 = mybir.dt.bfloat16
P = 128

SWAP_MASK = list(range(16, 32)) + list(range(16))  # half-swap in 4-row groups


@dataclass(frozen=True)
class Cfg:
    S: int = 2048      # sequence length
    DIM: int = 4096    # model dim (contraction for projections)
    NH_L: int = 8      # q heads per core
    NKV_L: int = 2     # kv heads per core
    HD: int = 128      # head dim (must be P)
    TQ: int = 512      # token/query chunk (PSUM free dim)

    @property
    def CCH(self):  # contraction chunks
        return self.DIM // P

    @property
    def NT(self):  # token chunks
        return self.S // self.TQ

    @property
    def NKT(self):  # key tiles
        return self.S // P

    @property
    def RT(self):  # key tiles per token chunk
        return self.TQ // P

    @property
    def NREP(self):
        return self.NH_L // self.NKV_L


def build_program(cfg: Cfg, debug: bool = False) -> bass.Bass:
    nc = bacc.Bacc("TRN2", target_bir_lowering=False)
    S, DIM, NH_L, NKV_L, HD, TQ = cfg.S, cfg.DIM, cfg.NH_L, cfg.NKV_L, cfg.HD, cfg.TQ
    CCH, NT, RT = cfg.CCH, cfg.NT, cfg.RT
    MULT = mybir.AluOpType.mult

    xT_d = nc.dram_tensor("xT", [DIM, S], F32R, kind="ExternalInput")
    wq_d = nc.dram_tensor("wq", [DIM, NH_L * HD], F32R, kind="ExternalInput")
    wk_d = nc.dram_tensor("wk", [DIM, NKV_L * HD], F32R, kind="ExternalInput")
    wv_d = nc.dram_tensor("wv", [DIM, NKV_L * HD], F32R, kind="ExternalInput")
    wo_d = nc.dram_tensor("wo", [NH_L * HD, DIM], F32R, kind="ExternalInput")
    cosq_d = nc.dram_tensor("cosq", [P, S], F32, kind="ExternalInput")
    sinq_d = nc.dram_tensor("sinq", [P, S], F32, kind="ExternalInput")
    cosk_d = nc.dram_tensor("cosk", [P, S], F32, kind="ExternalInput")
    sink_d = nc.dram_tensor("sink", [P, S], F32, kind="ExternalInput")
    pmask_d = nc.dram_tensor("pmask", [P, RT, TQ], BF16, kind="ExternalInput")
    out_d = nc.dram_tensor("out", [S, DIM], F32, kind="ExternalOutput")

    if debug:
        dbg_kt = nc.dram_tensor("dbg_kt", [P, NKV_L, S], F32, kind="ExternalOutput")
        dbg_v = nc.dram_tensor("dbg_v", [P, cfg.NKT, NKV_L * HD], BF16,
                               kind="ExternalOutput")
        dbg_qt = nc.dram_tensor("dbg_qt", [P, NH_L, S], F32, kind="ExternalOutput")
        dbg_at = nc.dram_tensor("dbg_at", [P, NH_L, S], F32, kind="ExternalOutput")

    xT_r = xT_d.ap().rearrange("(co ci) t -> ci co t", ci=P)
    wq_r = wq_d.ap().rearrange("(co ci) d -> ci co d", ci=P)
    wk_r = wk_d.ap().rearrange("(co ci) d -> ci co d", ci=P)
    wv_r = wv_d.ap().rearrange("(co ci) d -> ci co d", ci=P)
    wo_r = wo_d.ap().rearrange("(dc p) m -> p dc m", p=P)

    def r(ap):
        return ap if ap.dtype == F32R else ap.bitcast(F32R)

    def mm(out, lhsT, rhs, start, stop):
        nc.tensor.matmul(out, r(lhsT), r(rhs), start=start, stop=stop)

    def mmb(out, lhsT, rhs, start, stop):
        nc.tensor.matmul(out, lhsT, rhs, start=start, stop=stop)

    with tile.TileContext(nc) as tc, ExitStack() as top:
        const = top.enter_context(tc.tile_pool(name="const", bufs=1))
        pmask_sb = const.tile([P, RT, TQ], BF16)
        ones_row = const.tile([P, P], BF16)

        kvp = top.enter_context(tc.tile_pool(name="kvp", bufs=1))
        KT_sb = kvp.tile([P, NKV_L, S], F32)
        V_sb = kvp.tile([P, cfg.NKT, NKV_L * HD], BF16)
        qtp = tc.alloc_tile_pool(name="qtp", bufs=1)
        qt_sb = qtp.tile([P, NH_L, S], F32)
        # first wq piece, prefetched during phase A's tail
        wq0p = tc.alloc_tile_pool(name="wq0p", bufs=1)
        PCH = 4  # c-chunks per wq piece
        wq_piece0 = wq0p.tile([P, PCH, NH_L * HD], F32R)

        def rope_inplace(dst, cos_sl, sin_sl, tmp_pool):
            # dst [P, n] f32 in SBUF: dst = dst*cos + swap_halves(dst)*sin
            n = dst.shape[-1]
            tmp = tmp_pool.tile([P, TQ], F32, tag="ropetmp", name="ropetmp")
            t = tmp[:, :n]
            nc.vector.stream_shuffle(t, dst, SWAP_MASK)
            nc.vector.tensor_tensor(t.bitcast(F32R), t, sin_sl, MULT)
            nc.vector.tensor_tensor(dst.bitcast(F32R), dst, cos_sl, MULT)
            nc.vector.tensor_add(dst.bitcast(F32R), dst, t)

        # ---------------- Phase A: K^T and V projections (+ RoPE on K) -----
        with ExitStack() as ctx:
            wkvp = ctx.enter_context(tc.tile_pool(name="wkvp", bufs=1))
            ktab = ctx.enter_context(tc.tile_pool(name="ktab", bufs=2))
            xap = ctx.enter_context(tc.tile_pool(name="xap", bufs=10))
            rtp = ctx.enter_context(tc.tile_pool(name="rtp", bufs=2))
            pka = ctx.enter_context(tc.tile_pool(name="pka", bufs=2, space="PSUM"))
            pva = ctx.enter_context(tc.tile_pool(name="pva", bufs=1, space="PSUM"))

            wk_sb = wkvp.tile([P, CCH, NKV_L * HD], F32R)
            wv_sb = wkvp.tile([P, CCH, NKV_L * HD], F32R)

            LOOK = 3
            for tn in range(NT):
                tsl = slice(tn * TQ, (tn + 1) * TQ)
                psk = [pka.tile([P, TQ], F32, tag=f"psk{d}", name=f"psk{d}")
                       for d in range(NKV_L)]
                psv = [pva.tile([P, NKV_L * HD], F32, tag=f"psv{j}", name=f"psv{j}")
                       for j in range(RT)]
                for c in range(CCH):
                    if tn == 0:
                        # JIT weights, LOOK chunks ahead of use; wk issues on
                        # sync, wv on scalar (descriptor-gen is per-segment
                        # serial work on the issuing engine — spread it)
                        if c == 0:
                            for cc in range(LOOK + 1):
                                nc.sync.dma_start(wk_sb[:, cc, :], wk_r[:, cc, :])
                                nc.scalar.dma_start(wv_sb[:, cc, :], wv_r[:, cc, :])
                            nc.sync.dma_start(pmask_sb[:], pmask_d.ap())
                            nc.vector.memset(ones_row[:], 1.0)
                        elif c + LOOK < CCH:
                            cc = c + LOOK
                            nc.sync.dma_start(wk_sb[:, cc, :], wk_r[:, cc, :])
                            nc.scalar.dma_start(wv_sb[:, cc, :], wv_r[:, cc, :])
                    if tn == NT - 1 and c % 8 == 0:
                        # prefetch the first wq piece for phase Q (one chunk
                        # per issue, spread across the c-loop)
                        nc.sync.dma_start(wq_piece0[:, c // 8, :],
                                          wq_r[:, c // 8, :])
                    if c == 1:
                        # per-tn K rope tables (small, after the gating loads)
                        cosk_t = ktab.tile([P, TQ], F32, tag="ckt", name="ckt")
                        sink_t = ktab.tile([P, TQ], F32, tag="skt", name="skt")
                        nc.sync.dma_start(cosk_t[:], cosk_d.ap()[:, tsl])
                        nc.sync.dma_start(sink_t[:], sink_d.ap()[:, tsl])
                    xt = xap.tile([P, TQ], F32R, tag="xa", name="xa")
                    nc.gpsimd.dma_start(xt[:], xT_r[:, c, tsl])
                    st, sp = c == 0, c == CCH - 1
                    for d in range(NKV_L):
                        mm(psk[d][:], wk_sb[:, c, d * HD:(d + 1) * HD], xt[:], st, sp)
                    for j in range(RT):
                        mm(psv[j][:], xt[:, j * P:(j + 1) * P], wv_sb[:, c, :], st, sp)
                for j in range(RT):
                    nc.scalar.copy(V_sb[:, tn * RT + j, :], psv[j][:])
                for d in range(NKV_L):
                    nc.scalar.copy(KT_sb[:, d, tsl].bitcast(F32R), psk[d][:])
                    rope_inplace(KT_sb[:, d, tsl], cosk_t[:], sink_t[:], rtp)

        # ---------------- Phase Q: Q^T projection (+ RoPE on Q) ------------
        # 2-level accumulation, NACC=2 groups of 16 c-chunks; wq pieces of 4
        # chunks stream JIT (one batched dma each), piece 0 already resident.
        NACC = 2
        GC = CCH // NACC          # c-chunks per accumulation group (16)
        NPC = GC // PCH           # pieces per group (4)
        with ExitStack() as ctx:
            wqp = ctx.enter_context(tc.tile_pool(name="wqp", bufs=4))
            qtab = ctx.enter_context(tc.tile_pool(name="qtab", bufs=2))
            xqp = ctx.enter_context(tc.tile_pool(name="xqp", bufs=10))
            rtq = ctx.enter_context(tc.tile_pool(name="rtq", bufs=2))
            pqa = ctx.enter_context(tc.tile_pool(name="pqa", bufs=1, space="PSUM"))

            def fetch_piece(g, p):
                # one chunk per issue (the sync engine pays per segment;
                # spreading issues beats batching)
                if g == 0 and p == 0:
                    return wq_piece0
                piece = wqp.tile([P, PCH, NH_L * HD], F32R, tag="wqs", name="wqs")
                c0 = g * GC + p * PCH
                for i in range(PCH):
                    nc.sync.dma_start(piece[:, i, :], wq_r[:, c0 + i, :])
                return piece

            # piece schedule: (g, p) fetched at the (g, tn, ci) emission point
            # where it is ~1.5 piece-windows ahead of first use
            pieces = {(0, 0): wq_piece0}
            for g in range(NACC):
                for tn in range(NT):
                    tsl = slice(tn * TQ, (tn + 1) * TQ)
                    if g == 0 and tn == 0:
                        for p in range(1, NPC):
                            pieces[(0, p)] = fetch_piece(0, p)
                    if g == NACC - 1:
                        cq = qtab.tile([P, TQ], F32, tag="cqt", name="cqt")
                        sq = qtab.tile([P, TQ], F32, tag="sqt", name="sqt")
                        nc.sync.dma_start(cq[:], cosq_d.ap()[:, tsl])
                        nc.sync.dma_start(sq[:], sinq_d.ap()[:, tsl])
                    psq = [pqa.tile([P, TQ], F32, tag=f"psq{h}", name=f"psq{h}")
                           for h in range(NH_L)]
                    for ci in range(GC):
                        # prefetch next group's pieces during this group's
                        # last tn (one piece per PCH-chunk stride)
                        if g + 1 < NACC and tn == NT - 1 and ci % PCH == 0:
                            pieces[(g + 1, ci // PCH)] = fetch_piece(
                                g + 1, ci // PCH)
                        piece = pieces[(g, ci // PCH)]
                        col = ci % PCH
                        xt = xqp.tile([P, TQ], F32R, tag="xq", name="xq")
                        nc.gpsimd.dma_start(xt[:], xT_r[:, g * GC + ci, tsl])
                        st, sp = ci == 0, ci == GC - 1
                        for h in range(NH_L):
                            mm(psq[h][:], piece[:, col, h * HD:(h + 1) * HD],
                               xt[:], st, sp)
                    for h in range(NH_L):
                        if g == 0:
                            nc.scalar.copy(qt_sb[:, h, tsl].bitcast(F32R),
                                           psq[h][:])
                        else:
                            nc.vector.tensor_add(qt_sb[:, h, tsl].bitcast(F32R),
                                                 qt_sb[:, h, tsl], psq[h][:])
                        if g == NACC - 1:
                            rope_inplace(qt_sb[:, h, tsl], cq[:], sq[:], rtq)

        wq0p.release()

        if debug:
            nc.sync.dma_start(dbg_kt.ap(), KT_sb[:])
            nc.sync.dma_start(dbg_v.ap(), V_sb[:])
            nc.sync.dma_start(dbg_qt.ap(), qt_sb[:])

        # ---------------- Phase S: attention per head ----------------------
        # Software-pipelined: PV(kt) emitted after scores(kt+LP); single fused
        # multiplicative bf16 mask per diagonal key-tile; denominator epilogue
        # (ones_row matmul -> broadcast sums in PSUM -> reciprocal ->
        # normalize) deferred one block. wo slabs for phase W prefetch here.
        atp = tc.alloc_tile_pool(name="atp", bufs=1, side="right")
        attnT_sb = atp.tile([P, NH_L, S], F32)
        wop = tc.alloc_tile_pool(name="wop", bufs=4, side="right")

        def fetch_wo(mc, dh):
            slab = wop.tile([P, 4, TQ], F32R, tag="wos", name="wos")
            msl = slice(mc * TQ, (mc + 1) * TQ)
            for i in range(4):
                nc.gpsimd.dma_start(slab[:, i, :], wo_r[:, dh * 4 + i, msl])
            return slab

        with ExitStack() as ctx:
            ptp = ctx.enter_context(tc.tile_pool(name="ptp", bufs=6))
            acp = ctx.enter_context(tc.tile_pool(name="acp", bufs=3))
            bcp = ctx.enter_context(tc.tile_pool(name="bcp", bufs=2))
            psc = ctx.enter_context(tc.tile_pool(name="psc", bufs=4, space="PSUM"))
            pso = ctx.enter_context(tc.tile_pool(name="pso", bufs=3, space="PSUM"))
            pss = ctx.enter_context(tc.tile_pool(name="pss", bufs=1, space="PSUM"))

            LP = 3  # PV lookahead (score-matmuls emitted ahead of each PV)

            def epilogue(pend):
                h, qsl, ps_out, acc = pend
                ps_sum = pss.tile([P, TQ], F32, tag="pssum", name="pssum")
                mmb(ps_sum[:], ones_row[:], acc[:], True, True)
                bc_sb = bcp.tile([P, TQ], F32, tag="bcsb", name="bcsb")
                nc.vector.reciprocal_approx_fast(out=bc_sb[:], in_=ps_sum[:])
                nc.vector.tensor_tensor(attnT_sb[:, h, qsl].bitcast(F32R),
                                        ps_out[:], bc_sb[:], MULT)

            wo_fetched = []
            pending = None
            nblk = 0
            for qc in range(NT):
                for h in range(NH_L):
                    nblk += 1
                    if nblk >= NT * NH_L - 1 and len(wo_fetched) < 2:
                        wo_fetched.append(fetch_wo(0, len(wo_fetched)))
                    g = h // cfg.NREP
                    qsl = slice(qc * TQ, (qc + 1) * TQ)
                    ps_out = pso.tile([P, TQ], F32, tag="psout", name="psout")
                    acc = acp.tile([P, TQ], BF16, tag="acc", name="acc")
                    nkt = (qc + 1) * RT
                    pts = {}
                    for step in range(nkt + LP):
                        if step < nkt:
                            kt = step
                            ps_sc = psc.tile([P, TQ], F32, tag="pssc", name="pssc")
                            mm(ps_sc[:], KT_sb[:, g, kt * P:(kt + 1) * P],
                               qt_sb[:, h, qsl], True, True)
                            pt = ptp.tile([P, TQ], BF16, tag="pt", name="pt")
                            nc.scalar.activation(pt[:], ps_sc[:],
                                                 mybir.ActivationFunctionType.Exp)
                            if kt >= qc * RT:
                                j = kt - qc * RT
                                nc.vector.tensor_tensor(pt[:], pt[:],
                                                        pmask_sb[:, j, :], MULT)
                            if kt == 0:
                                nc.vector.tensor_copy(acc[:], pt[:])
                            else:
                                nc.vector.tensor_add(acc[:], acc[:], pt[:])
                            pts[kt] = pt
                        if step == min(2, nkt - 1) and pending is not None:
                            epilogue(pending)
                            pending = None
                        j = step - LP
                        if 0 <= j < nkt:
                            mmb(ps_out[:], V_sb[:, j, g * HD:(g + 1) * HD],
                                pts.pop(j)[:], j == 0, j == nkt - 1)
                    pending = (h, qsl, ps_out, acc)
            epilogue(pending)

        if debug:
            nc.sync.dma_start(dbg_at.ap(), attnT_sb[:])

        qtp.release()

        # ---------------- Phase W: output projection -----------------------
        with ExitStack() as ctx:
            owp = ctx.enter_context(tc.tile_pool(name="owp", bufs=3, side="right"))
            psw = ctx.enter_context(tc.tile_pool(name="psw", bufs=4, space="PSUM"))

            cur = wo_fetched
            for mc in range(DIM // TQ):
                msl = slice(mc * TQ, (mc + 1) * TQ)
                nxt = []
                for tb in range(S // P):
                    if mc + 1 < DIM // TQ and tb in (1, 3):
                        nxt.append(fetch_wo(mc + 1, len(nxt)))
                    ps_w = psw.tile([P, TQ], F32, tag="psw", name="psw")
                    for dc in range(NH_L):
                        mm(ps_w[:], attnT_sb[:, dc, tb * P:(tb + 1) * P],
                           cur[dc // 4][:, dc % 4, :], dc == 0, dc == NH_L - 1)
                    ot = owp.tile([P, TQ], F32, tag="ot", name="ot")
                    nc.scalar.copy(ot[:], ps_w[:])
                    nc.sync.dma_start(out_d.ap()[tb * P:(tb + 1) * P, msl], ot[:])
                if nxt:
                    cur = nxt

        wop.release()
        atp.release()

    nc.compile()
    return nc


# ---------------------------------------------------------------------------
# Host side
# ---------------------------------------------------------------------------

# Rope-pair layout: within each 32-partition quadrant q, rows q*32+[0:16) hold
# the even pair elements for frequency indices 16q+j and rows q*32+[16:32)
# the odd ones, so the rope partner swap is quadrant-local (stream_shuffle
# can only permute within 32-partition quadrants).
_QUAD_PERM = np.concatenate([
    np.concatenate([np.arange(0, P, 2)[q * 16:(q + 1) * 16],
                    np.arange(1, P, 2)[q * 16:(q + 1) * 16]])
    for q in range(4)
])
# row -> rope frequency index, and the sin sign per row
_FREQ_IDX = np.concatenate([np.tile(np.arange(16) + 16 * q, 2) for q in range(4)])
_SIN_SGN = np.tile(np.concatenate([-np.ones(16), np.ones(16)]), 4).astype(np.float32)

LAST_EXEC_NS = None
LAST_RESULTS = None


def _host_prep(cfg: Cfg, x, wq, wk, wv, wo, freqs_cos, freqs_sin):
    """Build the 8 per-core input maps. Core c: batch c % 2, group c // 2."""
    import ml_dtypes
    BF = ml_dtypes.bfloat16

    B = x.shape[0]
    n_groups = wq.shape[1] // (cfg.NH_L * cfg.HD)
    hd = cfg.HD

    cosT = np.ascontiguousarray(freqs_cos.T.astype(np.float32))  # [HD/2, S]
    sinT = np.ascontiguousarray(freqs_sin.T.astype(np.float32))
    sc = np.float32(1.0 / math.sqrt(hd))
    cosq = np.ascontiguousarray(cosT[_FREQ_IDX] * sc)
    sinq = np.ascontiguousarray(sinT[_FREQ_IDX] * _SIN_SGN[:, None] * sc)
    cosk = np.ascontiguousarray(cosT[_FREQ_IDX])
    sink = np.ascontiguousarray(sinT[_FREQ_IDX] * _SIN_SGN[:, None])
    # fused multiplicative causal masks for the transposed diagonal tiles:
    # variant j ([P, TQ]): cols < j*P -> 0, cols in [j*P,(j+1)*P) -> triu,
    # cols >= (j+1)*P -> 1   (allowed iff key k <= query q)
    pmask = np.ones((cfg.RT, P, cfg.TQ), np.float32)
    tri = np.triu(np.ones((P, P), np.float32), 0)
    for j in range(cfg.RT):
        pmask[j, :, : j * P] = 0.0
        pmask[j, :, j * P:(j + 1) * P] = tri
    pmask = np.ascontiguousarray(pmask.transpose(1, 0, 2)).astype(BF)  # [P,RT,TQ]

    xT = [np.ascontiguousarray(x[b].T).astype(np.float32) for b in range(B)]

    def permute_cols(w, nheads):
        w = w.reshape(cfg.DIM, nheads, hd)[:, :, _QUAD_PERM]
        return np.ascontiguousarray(w.reshape(cfg.DIM, nheads * hd), dtype=np.float32)

    in_maps = []
    qcols = cfg.NH_L * hd
    kcols = cfg.NKV_L * hd
    for c in range(B * n_groups):
        b, g = c % B, c // B
        in_maps.append(dict(
            xT=xT[b],
            wq=permute_cols(wq[:, g * qcols:(g + 1) * qcols], cfg.NH_L),
            wk=permute_cols(wk[:, g * kcols:(g + 1) * kcols], cfg.NKV_L),
            wv=np.ascontiguousarray(wv[:, g * kcols:(g + 1) * kcols], dtype=np.float32),
            wo=np.ascontiguousarray(wo[g * qcols:(g + 1) * qcols, :], dtype=np.float32),
            cosq=cosq, sinq=sinq, cosk=cosk, sink=sink, pmask=pmask,
        ))
    return in_maps


def kernel(x, wq, wk, wv, wo, freqs_cos, freqs_sin, mask, start_pos=0):
    global LAST_EXEC_NS, LAST_RESULTS
    x = np.asarray(x, np.float32)
    wq = np.asarray(wq, np.float32)
    wk = np.asarray(wk, np.float32)
    wv = np.asarray(wv, np.float32)
    wo = np.asarray(wo, np.float32)
    freqs_cos = np.asarray(freqs_cos, np.float32)
    freqs_sin = np.asarray(freqs_sin, np.float32)

    cfg = Cfg()
    B = x.shape[0]
    n_groups = 4
    in_maps = _host_prep(cfg, x, wq, wk, wv, wo, freqs_cos, freqs_sin)

    from concourse.bass_utils import run_bass_kernel_spmd

    debug = bool(int(os.environ.get("KERNEL_DEBUG", "0")))
    nc = build_program(cfg, debug=debug)
    trace = bool(int(os.environ.get("KERNEL_TRACE", "0")))
    res = run_bass_kernel_spmd(nc, in_maps, core_ids=list(range(len(in_maps))),
                               trace=trace)
    LAST_EXEC_NS = res.exec_time_ns
    LAST_RESULTS = res

    out = np.zeros((B, cfg.S, cfg.DIM), np.float32)
    for c in range(B * n_groups):
        b = c % B
        out[b] += res.results[c]["out"]
    return out


# revision 33
# speedup vs baseline: 1.0971x; 1.0269x over previous
"""GQA attention block (B=2, S=2048, DIM=4096, 32 Q heads / 8 KV heads, HD=128,
RoPE + causal softmax + output projection) on 8 trn2 NeuronCores.

Sharding: 8 cores = 2 batches x 4 head-groups. Core c handles batch c%2 and
head-group c//2 (8 Q heads, 2 KV heads). Each core computes a full-size
[S, DIM] partial of the output projection (its heads' contribution); the host
sums the 4 group-partials per batch.

v4: v3 (hybrid fp32r/bf16, software-pipelined attention) + sync-engine relief.
The sync sequencer pays ~0.6us per dma_start; v3 lost ~38us at the A->Q seam
to a burst of single-chunk issues. v4:
  - batches every weight/x stream into multi-chunk dma_starts,
  - replaces the RoPE half-swap SBUF-SBUF DMA pair with one
    vector.stream_shuffle,
  - prefetches the first wq piece during phase A and the first wo slabs
    during phase S,
  - fuses the causal mask into one precomputed multiplicative bf16 tile per
    diagonal offset (single DVE op per diagonal key-tile).
"""

import math
import os
import sys
from contextlib import ExitStack
from dataclasses import dataclass

import numpy as np

sys.path.insert(0, "/opt/trn_rl_repo")

import concourse.bass as bass  # noqa: E402
import concourse.mybir as mybir  # noqa: E402
import concourse.tile as tile  # noqa: E402
from concourse import bacc  # noqa: E402

F32 = mybir.dt.float32
F32R = mybir.dt.float32r
BF16 = mybir.dt.bfloat16
P = 128

SWAP_MASK = list(range(16, 32)) + list(range(16))  # half-swap in 4-row groups


@dataclass(frozen=True)
class Cfg:
    S: int = 2048      # sequence length
    DIM: int = 4096    # model dim (contraction for projections)
    NH_L: int = 8      # q heads per core
    NKV_L: int = 2     # kv heads per core
    HD: int = 128      # head dim (must be P)
    TQ: int = 512      # token/query chunk (PSUM free dim)

    @property
    def CCH(self):  # contraction chunks
        return self.DIM // P

    @property
    def NT(self):  # token chunks
        return self.S // self.TQ

    @property
    def NKT(self):  # key tiles
        return self.S // P

    @property
    def RT(self):  # key tiles per token chunk
        return self.TQ // P

    @property
    def NREP(self):
        return self.NH_L // self.NKV_L


def build_program(cfg: Cfg, debug: bool = False) -> bass.Bass:
    nc = bacc.Bacc("TRN2", target_bir_lowering=False)
    S, DIM, NH_L, NKV_L, HD, TQ = cfg.S, cfg.DIM, cfg.NH_L, cfg.NKV_L, cfg.HD, cfg.TQ
    CCH, NT, RT = cfg.CCH, cfg.NT, cfg.RT
    MULT = mybir.AluOpType.mult

    xT_d = nc.dram_tensor("xT", [DIM, S], F32R, kind="ExternalInput")
    wq_d = nc.dram_tensor("wq", [DIM, NH_L * HD], F32R, kind="ExternalInput")
    wk_d = nc.dram_tensor("wk", [DIM, NKV_L * HD], F32R, kind="ExternalInput")
    wv_d = nc.dram_tensor("wv", [DIM, NKV_L * HD], F32R, kind="ExternalInput")
    wo_d = nc.dram_tensor("wo", [NH_L * HD, DIM], F32R, kind="ExternalInput")
    cosq_d = nc.dram_tensor("cosq", [P, S], F32, kind="ExternalInput")
    sinq_d = nc.dram_tensor("sinq", [P, S], F32, kind="ExternalInput")
    cosk_d = nc.dram_tensor("cosk", [P, S], F32, kind="ExternalInput")
    sink_d = nc.dram_tensor("sink", [P, S], F32, kind="ExternalInput")
    pmask_d = nc.dram_tensor("pmask", [P, RT, TQ], BF16, kind="ExternalInput")
    out_d = nc.dram_tensor("out", [S, DIM], F32, kind="ExternalOutput")

    if debug:
        dbg_kt = nc.dram_tensor("dbg_kt", [P, NKV_L, S], F32, kind="ExternalOutput")
        dbg_v = nc.dram_tensor("dbg_v", [P, cfg.NKT, NKV_L * HD], BF16,
                               kind="ExternalOutput")
        dbg_qt = nc.dram_tensor("dbg_qt", [P, NH_L, S], F32, kind="ExternalOutput")
        dbg_at = nc.dram_tensor("dbg_at", [P, NH_L, S], F32, kind="ExternalOutput")

    xT_r = xT_d.ap().rearrange("(co ci) t -> ci co t", ci=P)
    wq_r = wq_d.ap().rearrange("(co ci) d -> ci co d", ci=P)
    wk_r = wk_d.ap().rearrange("(co ci) d -> ci co d", ci=P)
    wv_r = wv_d.ap().rearrange("(co ci) d -> ci co d", ci=P)
    wo_r = wo_d.ap().rearrange("(dc p) m -> p dc m", p=P)

    def r(ap):
        return ap if ap.dtype == F32R else ap.bitcast(F32R)

    def mm(out, lhsT, rhs, start, stop):
        nc.tensor.matmul(out, r(lhsT), r(rhs), start=start, stop=stop)

    def mmb(out, lhsT, rhs, start, stop):
        nc.tensor.matmul(out, lhsT, rhs, start=start, stop=stop)

    with tile.TileContext(nc) as tc, ExitStack() as top:
        const = top.enter_context(tc.tile_pool(name="const", bufs=1))
        pmask_sb = const.tile([P, RT, TQ], BF16)
        ones_row = const.tile([P, P], BF16)

        kvp = top.enter_context(tc.tile_pool(name="kvp", bufs=1))
        KT_sb = kvp.tile([P, NKV_L, S], F32)
        V_sb = kvp.tile([P, cfg.NKT, NKV_L * HD], BF16)
        qtp = tc.alloc_tile_pool(name="qtp", bufs=1)
        qt_sb = qtp.tile([P, NH_L, S], F32)
        # first wq piece, prefetched during phase A's tail
        wq0p = tc.alloc_tile_pool(name="wq0p", bufs=1)
        PCH = 4  # c-chunks per wq piece
        wq_piece0 = wq0p.tile([P, PCH, NH_L * HD], F32R)

        def rope_inplace(dst, cos_sl, sin_sl, tmp_pool):
            # dst [P, n] f32 in SBUF: dst = dst*cos + swap_halves(dst)*sin
            n = dst.shape[-1]
            tmp = tmp_pool.tile([P, TQ], F32, tag="ropetmp", name="ropetmp")
            t = tmp[:, :n]
            nc.vector.stream_shuffle(t, dst, SWAP_MASK)
            nc.vector.tensor_tensor(t.bitcast(F32R), t, sin_sl, MULT)
            nc.vector.tensor_tensor(dst.bitcast(F32R), dst, cos_sl, MULT)
            nc.vector.tensor_add(dst.bitcast(F32R), dst, t)

        # ---------------- Phase A: K^T and V projections (+ RoPE on K) -----
        with ExitStack() as ctx:
            wkvp = ctx.enter_context(tc.tile_pool(name="wkvp", bufs=1))
            ktab = ctx.enter_context(tc.tile_pool(name="ktab", bufs=2))
            xap = ctx.enter_context(tc.tile_pool(name="xap", bufs=10))
            rtp = ctx.enter_context(tc.tile_pool(name="rtp", bufs=2))
            pka = ctx.enter_context(tc.tile_pool(name="pka", bufs=2, space="PSUM"))
            pva = ctx.enter_context(tc.tile_pool(name="pva", bufs=1, space="PSUM"))

            wk_sb = wkvp.tile([P, CCH, NKV_L * HD], F32R)
            wv_sb = wkvp.tile([P, CCH, NKV_L * HD], F32R)

            LOOK = 3
            for tn in range(NT):
                tsl = slice(tn * TQ, (tn + 1) * TQ)
                psk = [pka.tile([P, TQ], F32, tag=f"psk{d}", name=f"psk{d}")
                       for d in range(NKV_L)]
                psv = [pva.tile([P, NKV_L * HD], F32, tag=f"psv{j}", name=f"psv{j}")
                       for j in range(RT)]
                for c in range(CCH):
                    if tn == 0:
                        # JIT weights, LOOK chunks ahead of use; wk issues on
                        # sync, wv on scalar (descriptor-gen is per-segment
                        # serial work on the issuing engine — spread it)
                        if c == 0:
                            for cc in range(LOOK + 1):
                                nc.sync.dma_start(wk_sb[:, cc, :], wk_r[:, cc, :])
                                nc.scalar.dma_start(wv_sb[:, cc, :], wv_r[:, cc, :])
                            nc.sync.dma_start(pmask_sb[:], pmask_d.ap())
                            nc.vector.memset(ones_row[:], 1.0)
                        elif c + LOOK < CCH:
                            cc = c + LOOK
                            nc.sync.dma_start(wk_sb[:, cc, :], wk_r[:, cc, :])
                            nc.scalar.dma_start(wv_sb[:, cc, :], wv_r[:, cc, :])
                    if tn == NT - 1 and c % 8 == 0:
                        # prefetch the first wq piece for phase Q (one chunk
                        # per issue, spread across the c-loop)
                        nc.sync.dma_start(wq_piece0[:, c // 8, :],
                                          wq_r[:, c // 8, :])
                    if c == 1:
                        # per-tn K rope tables (small, after the gating loads)
                        cosk_t = ktab.tile([P, TQ], F32, tag="ckt", name="ckt")
                        sink_t = ktab.tile([P, TQ], F32, tag="skt", name="skt")
                        nc.sync.dma_start(cosk_t[:], cosk_d.ap()[:, tsl])
                        nc.sync.dma_start(sink_t[:], sink_d.ap()[:, tsl])
                    xt = xap.tile([P, TQ], F32R, tag="xa", name="xa")
                    nc.gpsimd.dma_start(xt[:], xT_r[:, c, tsl])
                    st, sp = c == 0, c == CCH - 1
                    for d in range(NKV_L):
                        mm(psk[d][:], wk_sb[:, c, d * HD:(d + 1) * HD], xt[:], st, sp)
                    for j in range(RT):
                        mm(psv[j][:], xt[:, j * P:(j + 1) * P], wv_sb[:, c, :], st, sp)
                for j in range(RT):
                    nc.scalar.copy(V_sb[:, tn * RT + j, :], psv[j][:])
                for d in range(NKV_L):
                    nc.scalar.copy(KT_sb[:, d, tsl].bitcast(F32R), psk[d][:])
                    rope_inplace(KT_sb[:, d, tsl], cosk_t[:], sink_t[:], rtp)

        # ---------------- Phase Q: Q^T projection (+ RoPE on Q) ------------
        # 2-level accumulation, NACC=2 groups of 16 c-chunks; wq pieces of 4
        # chunks stream JIT (one batched dma each), piece 0 already resident.
        NACC = 2
        GC = CCH // NACC          # c-chunks per accumulation group (16)
        NPC = GC // PCH           # pieces per group (4)
        with ExitStack() as ctx:
            wqp = ctx.enter_context(tc.tile_pool(name="wqp", bufs=4))
            qtab = ctx.enter_context(tc.tile_pool(name="qtab", bufs=2))
            xqp = ctx.enter_context(tc.tile_pool(name="xqp", bufs=10))
            rtq = ctx.enter_context(tc.tile_pool(name="rtq", bufs=2))
            pqa = ctx.enter_context(tc.tile_pool(name="pqa", bufs=1, space="PSUM"))

            def fetch_piece(g, p):
                # one chunk per issue (the sync engine pays per segment;
                # spreading issues beats batching)
                if g == 0 and p == 0:
                    return wq_piece0
                piece = wqp.tile([P, PCH, NH_L * HD], F32R, tag="wqs", name="wqs")
                c0 = g * GC + p * PCH
                for i in range(PCH):
                    nc.sync.dma_start(piece[:, i, :], wq_r[:, c0 + i, :])
                return piece

            # piece schedule: (g, p) fetched at the (g, tn, ci) emission point
            # where it is ~1.5 piece-windows ahead of first use
            pieces = {(0, 0): wq_piece0}
            for g in range(NACC):
                for tn in range(NT):
                    tsl = slice(tn * TQ, (tn + 1) * TQ)
                    if g == 0 and tn == 0:
                        for p in range(1, NPC):
                            pieces[(0, p)] = fetch_piece(0, p)
                    if g == NACC - 1:
                        cq = qtab.tile([P, TQ], F32, tag="cqt", name="cqt")
                        sq = qtab.tile([P, TQ], F32, tag="sqt", name="sqt")
                        nc.sync.dma_start(cq[:], cosq_d.ap()[:, tsl])
                        nc.sync.dma_start(sq[:], sinq_d.ap()[:, tsl])
                    psq = [pqa.tile([P, TQ], F32, tag=f"psq{h}", name=f"psq{h}")
                           for h in range(NH_L)]
                    for ci in range(GC):
                        # prefetch next group's pieces during this group's
                        # last tn (one piece per PCH-chunk stride)
                        if g + 1 < NACC and tn == NT - 1 and ci % PCH == 0:
                            pieces[(g + 1, ci // PCH)] = fetch_piece(
                                g + 1, ci // PCH)
                        piece = pieces[(g, ci // PCH)]
                        col = ci % PCH
                        xt = xqp.tile([P, TQ], F32R, tag="xq", name="xq")
                        nc.gpsimd.dma_start(xt[:], xT_r[:, g * GC + ci, tsl])
                        st, sp = ci == 0, ci == GC - 1
                        for h in range(NH_L):
                            mm(psq[h][:], piece[:, col, h * HD:(h + 1) * HD],
                               xt[:], st, sp)
                    # drain all psq banks first (frees PSUM for the next tn /
                    # phase S), then rope — the ropes are 4 vector ops each
                    # and must not delay the bank frees
                    for h in range(NH_L):
                        if g == 0:
                            nc.scalar.copy(qt_sb[:, h, tsl].bitcast(F32R),
                                           psq[h][:])
                        else:
                            nc.vector.tensor_add(qt_sb[:, h, tsl].bitcast(F32R),
                                                 qt_sb[:, h, tsl], psq[h][:])
                    if g == NACC - 1:
                        for h in range(NH_L):
                            rope_inplace(qt_sb[:, h, tsl], cq[:], sq[:], rtq)

        wq0p.release()

        if debug:
            nc.sync.dma_start(dbg_kt.ap(), KT_sb[:])
            nc.sync.dma_start(dbg_v.ap(), V_sb[:])
            nc.sync.dma_start(dbg_qt.ap(), qt_sb[:])

        # ---------------- Phase S: attention per head ----------------------
        # Software-pipelined: PV(kt) emitted after scores(kt+LP); single fused
        # multiplicative bf16 mask per diagonal key-tile; denominator epilogue
        # (ones_row matmul -> broadcast sums in PSUM -> reciprocal ->
        # normalize) deferred one block. wo slabs for phase W prefetch here.
        atp = tc.alloc_tile_pool(name="atp", bufs=1, side="right")
        attnT_sb = atp.tile([P, NH_L, S], F32)
        wop = tc.alloc_tile_pool(name="wop", bufs=4, side="right")

        def fetch_wo(mc, dh):
            slab = wop.tile([P, 4, TQ], F32R, tag="wos", name="wos")
            msl = slice(mc * TQ, (mc + 1) * TQ)
            for i in range(4):
                nc.gpsimd.dma_start(slab[:, i, :], wo_r[:, dh * 4 + i, msl])
            return slab

        with ExitStack() as ctx:
            ptp = ctx.enter_context(tc.tile_pool(name="ptp", bufs=6))
            acp = ctx.enter_context(tc.tile_pool(name="acp", bufs=3))
            bcp = ctx.enter_context(tc.tile_pool(name="bcp", bufs=2))
            psc = ctx.enter_context(tc.tile_pool(name="psc", bufs=4, space="PSUM"))
            pso = ctx.enter_context(tc.tile_pool(name="pso", bufs=3, space="PSUM"))
            pss = ctx.enter_context(tc.tile_pool(name="pss", bufs=1, space="PSUM"))

            LP = 3  # PV lookahead (score-matmuls emitted ahead of each PV)

            def epilogue(pend):
                h, qsl, ps_out, acc = pend
                ps_sum = pss.tile([P, TQ], F32, tag="pssum", name="pssum")
                mmb(ps_sum[:], ones_row[:], acc[:], True, True)
                bc_sb = bcp.tile([P, TQ], F32, tag="bcsb", name="bcsb")
                nc.vector.reciprocal_approx_fast(out=bc_sb[:], in_=ps_sum[:])
                nc.vector.tensor_tensor(attnT_sb[:, h, qsl].bitcast(F32R),
                                        ps_out[:], bc_sb[:], MULT)

            # flat cross-block software pipeline: one global stream of
            # score-steps; each PV trails its score by LP steps, crossing
            # block boundaries so the PE has no per-block tail bubble.
            blocks = [(qc, h) for qc in range(NT) for h in range(NH_L)]
            steps = [(bi, kt)
                     for bi, (qc, h) in enumerate(blocks)
                     for kt in range((qc + 1) * RT)]
            bstate = {}
            pts = {}
            wo_fetched = []
            for i in range(len(steps) + LP):
                if i < len(steps):
                    bi, kt = steps[i]
                    qc, h = blocks[bi]
                    if kt == 0:
                        bstate[bi] = (
                            pso.tile([P, TQ], F32, tag="psout", name="psout"),
                            acp.tile([P, TQ], BF16, tag="acc", name="acc"),
                        )
                        if bi >= len(blocks) - 2 and len(wo_fetched) < 2:
                            wo_fetched.append(fetch_wo(0, len(wo_fetched)))
                    ps_out, acc = bstate[bi]
                    g = h // cfg.NREP
                    qsl = slice(qc * TQ, (qc + 1) * TQ)
                    ps_sc = psc.tile([P, TQ], F32, tag="pssc", name="pssc")
                    mm(ps_sc[:], KT_sb[:, g, kt * P:(kt + 1) * P],
                       qt_sb[:, h, qsl], True, True)
                    pt = ptp.tile([P, TQ], BF16, tag="pt", name="pt")
                    nc.scalar.activation(pt[:], ps_sc[:],
                                         mybir.ActivationFunctionType.Exp)
                    if kt >= qc * RT:
                        nc.vector.tensor_tensor(pt[:], pt[:],
                                                pmask_sb[:, kt - qc * RT, :],
                                                MULT)
                    if kt == 0:
                        nc.vector.tensor_copy(acc[:], pt[:])
                    else:
                        nc.vector.tensor_add(acc[:], acc[:], pt[:])
                    pts[(bi, kt)] = pt
                j = i - LP
                if j >= 0:
                    bj, ktj = steps[j]
                    qcj, hj = blocks[bj]
                    gj = hj // cfg.NREP
                    nktj = (qcj + 1) * RT
                    ps_out, acc = bstate[bj]
                    mmb(ps_out[:], V_sb[:, ktj, gj * HD:(gj + 1) * HD],
                        pts.pop((bj, ktj))[:], ktj == 0, ktj == nktj - 1)
                    if ktj == nktj - 1:
                        qslj = slice(qcj * TQ, (qcj + 1) * TQ)
                        epilogue((hj, qslj, ps_out, acc))
                        del bstate[bj]

        if debug:
            nc.sync.dma_start(dbg_at.ap(), attnT_sb[:])

        qtp.release()

        # ---------------- Phase W: output projection -----------------------
        with ExitStack() as ctx:
            owp = ctx.enter_context(tc.tile_pool(name="owp", bufs=3, side="right"))
            psw = ctx.enter_context(tc.tile_pool(name="psw", bufs=4, space="PSUM"))

            cur = wo_fetched
            for mc in range(DIM // TQ):
                msl = slice(mc * TQ, (mc + 1) * TQ)
                nxt = []
                for tb in range(S // P):
                    if mc + 1 < DIM // TQ and tb in (1, 3):
                        nxt.append(fetch_wo(mc + 1, len(nxt)))
                    ps_w = psw.tile([P, TQ], F32, tag="psw", name="psw")
                    for dc in range(NH_L):
                        mm(ps_w[:], attnT_sb[:, dc, tb * P:(tb + 1) * P],
                           cur[dc // 4][:, dc % 4, :], dc == 0, dc == NH_L - 1)
                    ot = owp.tile([P, TQ], F32, tag="ot", name="ot")
                    nc.scalar.copy(ot[:], ps_w[:])
                    nc.sync.dma_start(out_d.ap()[tb * P:(tb + 1) * P, msl], ot[:])
                if nxt:
                    cur = nxt

        wop.release()
        atp.release()

    nc.compile()
    return nc


# ---------------------------------------------------------------------------
# Host side
# ---------------------------------------------------------------------------

# Rope-pair layout: within each 32-partition quadrant q, rows q*32+[0:16) hold
# the even pair elements for frequency indices 16q+j and rows q*32+[16:32)
# the odd ones, so the rope partner swap is quadrant-local (stream_shuffle
# can only permute within 32-partition quadrants).
_QUAD_PERM = np.concatenate([
    np.concatenate([np.arange(0, P, 2)[q * 16:(q + 1) * 16],
                    np.arange(1, P, 2)[q * 16:(q + 1) * 16]])
    for q in range(4)
])
# row -> rope frequency index, and the sin sign per row
_FREQ_IDX = np.concatenate([np.tile(np.arange(16) + 16 * q, 2) for q in range(4)])
_SIN_SGN = np.tile(np.concatenate([-np.ones(16), np.ones(16)]), 4).astype(np.float32)

LAST_EXEC_NS = None
LAST_RESULTS = None


def _host_prep(cfg: Cfg, x, wq, wk, wv, wo, freqs_cos, freqs_sin):
    """Build the 8 per-core input maps. Core c: batch c % 2, group c // 2."""
    import ml_dtypes
    BF = ml_dtypes.bfloat16

    B = x.shape[0]
    n_groups = wq.shape[1] // (cfg.NH_L * cfg.HD)
    hd = cfg.HD

    cosT = np.ascontiguousarray(freqs_cos.T.astype(np.float32))  # [HD/2, S]
    sinT = np.ascontiguousarray(freqs_sin.T.astype(np.float32))
    sc = np.float32(1.0 / math.sqrt(hd))
    cosq = np.ascontiguousarray(cosT[_FREQ_IDX] * sc)
    sinq = np.ascontiguousarray(sinT[_FREQ_IDX] * _SIN_SGN[:, None] * sc)
    cosk = np.ascontiguousarray(cosT[_FREQ_IDX])
    sink = np.ascontiguousarray(sinT[_FREQ_IDX] * _SIN_SGN[:, None])
    # fused multiplicative causal masks for the transposed diagonal tiles:
    # variant j ([P, TQ]): cols < j*P -> 0, cols in [j*P,(j+1)*P) -> triu,
    # cols >= (j+1)*P -> 1   (allowed iff key k <= query q)
    pmask = np.ones((cfg.RT, P, cfg.TQ), np.float32)
    tri = np.triu(np.ones((P, P), np.float32), 0)
    for j in range(cfg.RT):
        pmask[j, :, : j * P] = 0.0
        pmask[j, :, j * P:(j + 1) * P] = tri
    pmask = np.ascontiguousarray(pmask.transpose(1, 0, 2)).astype(BF)  # [P,RT,TQ]

    xT = [np.ascontiguousarray(x[b].T).astype(np.float32) for b in range(B)]

    def permute_cols(w, nheads):
        w = w.reshape(cfg.DIM, nheads, hd)[:, :, _QUAD_PERM]
        return np.ascontiguousarray(w.reshape(cfg.DIM, nheads * hd), dtype=np.float32)

    in_maps = []
    qcols = cfg.NH_L * hd
    kcols = cfg.NKV_L * hd
    for c in range(B * n_groups):
        b, g = c % B, c // B
        in_maps.append(dict(
            xT=xT[b],
            wq=permute_cols(wq[:, g * qcols:(g + 1) * qcols], cfg.NH_L),
            wk=permute_cols(wk[:, g * kcols:(g + 1) * kcols], cfg.NKV_L),
            wv=np.ascontiguousarray(wv[:, g * kcols:(g + 1) * kcols], dtype=np.float32),
            wo=np.ascontiguousarray(wo[g * qcols:(g + 1) * qcols, :], dtype=np.float32),
            cosq=cosq, sinq=sinq, cosk=cosk, sink=sink, pmask=pmask,
        ))
    return in_maps


def kernel(x, wq, wk, wv, wo, freqs_cos, freqs_sin, mask, start_pos=0):
    global LAST_EXEC_NS, LAST_RESULTS
    x = np.asarray(x, np.float32)
    wq = np.asarray(wq, np.float32)
    wk = np.asarray(wk, np.float32)
    wv = np.asarray(wv, np.float32)
    wo = np.asarray(wo, np.float32)
    freqs_cos = np.asarray(freqs_cos, np.float32)
    freqs_sin = np.asarray(freqs_sin, np.float32)

    cfg = Cfg()
    B = x.shape[0]
    n_groups = 4
    in_maps = _host_prep(cfg, x, wq, wk, wv, wo, freqs_cos, freqs_sin)

    from concourse.bass_utils import run_bass_kernel_spmd

    debug = bool(int(os.environ.get("KERNEL_DEBUG", "0")))
    nc = build_program(cfg, debug=debug)
    trace = bool(int(os.environ.get("KERNEL_TRACE", "0")))
    res = run_bass_kernel_spmd(nc, in_maps, core_ids=list(range(len(in_maps))),
                               trace=trace)
    LAST_EXEC_NS = res.exec_time_ns
    LAST_RESULTS = res

    out = np.zeros((B, cfg.S, cfg.DIM), np.float32)
    for c in range(B * n_groups):
        b = c % B
        out[b] += res.results[c]["out"]
    return out
